# revision 71
# baseline (speedup 1.0000x reference)
"""Trainium2 Bass kernel for nn_DGNN (gnn_message_passing).

Reference computation (B=4, N=8192, F=32):
    delay_steps = time_delay // 5
    active      = (t >= delay_steps) & (adj > 0)
    A           = where(active, adj, 0)              # == adj * (time_delay <= 5*t+4)
    adjusted    = einsum('ij,bjf->bif', A, x)
    h           = relu(adjusted @ W1 + b1)
    out         = sigmoid(h @ W2 + b2)

Sharding / layout (host does layout-only transforms, no reference math):
  - destination nodes i are split row-wise across 8 cores (1024 each);
  - adj/time_delay are shipped transposed ([j, i], j on partitions) because
    the PE contracts over the partition dim; both are additionally packed so
    that each DMA chunk (4 contraction tiles for adj, 8 for td) is one
    fully-contiguous ~1 MB block -> few fat descriptors, ~26 dma_starts
    per core instead of ~140;
  - adj is shipped as fp16 (lossy container narrowing, ~7e-3 output rel err
    vs the 2e-2 gate) and time_delay as int8 (lossless: values 0..99);
  - x is repacked so the 4 batches sit side-by-side in the stationary
    operand (partition q = 32*b + f), giving full-width M=128 matmuls;
  - W1/W2 become 128x128 block-diagonal so the per-node MLP handles all 4
    batches in one matmul.

On-device per core: stream adj/td chunks and build the masked adjacency
A = (td <= thr) * adj two tiles at a time, strictly alternating two paths
so no single engine serializes the pipeline:
  - DVE path: one fused scalar_tensor_tensor (is_le + mult; 1x mode - the
    int8 operand blocks the DVE's 16-bit 2x mode);
  - ACT path: the Scalar engine computes m = 1[td <= thr] as a saturated
    sigmoid (min |60*(thr+0.5-td)| = 30, so fp16 m is exactly 0 or 1),
    then an all-fp16 DVE tensor_tensor multiply that runs in 2x mode.
fp16 matmuls (1 cycle/row) accumulate adjusted^T over 64 K-tiles in fp32
PSUM, then the block-diagonal MLP and sigmoid run on-chip in pipelined
column quarters. A small primer super-chunk at sc=0 gets the first matmul
going within a few us. Output returns transposed per core and is unsharded
on the host. The kernel is DMA-bound: ~26 MB/core of HBM reads at
~330-390 GB/s. mm_dtype_name="float32" keeps an all-fp32 fallback path.
"""

import numpy as np

B = 4
N = 8192
F = 32
P = 128
NCORES = 8
NI = N // NCORES  # dest-nodes per core
JT = N // P       # contraction tiles

MM_N = 512        # moving-operand free dim per matmul (one PSUM bank)
KA = 4            # adj tiles per DMA chunk
KT = 8            # td tiles per DMA chunk


def _build(nj, ni, thr, mm_dtype_name="float16", td_dtype=np.int8):
    """Trace + compile the per-core Bass program."""
    from contextlib import ExitStack

    import concourse.bacc as bacc
    import concourse.mybir as mybir
    import concourse.tile as tile

    f32 = mybir.dt.float32
    mm_dt = getattr(mybir.dt, mm_dtype_name)
    adj_dt = mm_dt if mm_dtype_name == "float16" else f32
    td_dt = mybir.dt.from_np(np.dtype(td_dtype))

    jt_n = nj // P
    mm_n = min(MM_N, ni)
    nh = ni // mm_n
    n_sc = jt_n // KT           # super-chunks (td granularity)
    na_per_sc = KT // KA        # adj chunks per super-chunk

    nc = bacc.Bacc("TRN2", target_bir_lowering=False, debug=False)

    adjP_d = nc.dram_tensor(
        "adjP", [(jt_n // KA) * P, KA * ni], adj_dt, kind="ExternalInput"
    ).ap()
    tdP_d = nc.dram_tensor(
        "tdP", [(jt_n // KT) * P, KT * ni], td_dt, kind="ExternalInput"
    ).ap()
    xsb_d = nc.dram_tensor("xsb", [P, jt_n * P], mm_dt, kind="ExternalInput").ap()
    bd1_d = nc.dram_tensor("bd1", [P, P], mm_dt, kind="ExternalInput").ap()
    bd2_d = nc.dram_tensor("bd2", [P, P], mm_dt, kind="ExternalInput").ap()
    bias1_d = nc.dram_tensor("bias1", [P, 1], f32, kind="ExternalInput").ap()
    bias2_d = nc.dram_tensor("bias2", [P, 1], f32, kind="ExternalInput").ap()
    # output ships fp16 (sigmoid range [0,1]; host widens to fp32)
    out_dt = mm_dt if mm_dtype_name == "float16" else f32
    outT_d = nc.dram_tensor("outT", [P, ni], out_dt, kind="ExternalOutput").ap()

    with tile.TileContext(nc) as tc, ExitStack() as ctx:
        adjp = ctx.enter_context(tc.tile_pool(name="adjp", bufs=9))
        tdp = ctx.enter_context(tc.tile_pool(name="tdp", bufs=4))
        wrk = ctx.enter_context(tc.tile_pool(name="wrk", bufs=10))
        mp = ctx.enter_context(tc.tile_pool(name="mp", bufs=7))
        singles = ctx.enter_context(tc.tile_pool(name="singles", bufs=1))
        pp = ctx.enter_context(tc.tile_pool(name="pp", bufs=1, space="PSUM"))

        x_t = singles.tile([P, jt_n * P], mm_dt)
        psum_main = pp.tile([P, ni], f32)
        scr_ps = pp.tile([P, mm_n], f32, tag="scr")
        bd1_t = singles.tile([P, P], mm_dt)
        bd2_t = singles.tile([P, P], mm_dt)
        bias1_t = singles.tile([P, 1], f32)
        bias2_t = singles.tile([P, 1], f32)
        warm_t = singles.tile([P, 1], f32)

        thr_f = float(thr)
        pair_ctr = [0]
        # per-partition scalars for the mask sigmoid: m = sigmoid(-60*td + b)
        mscale_t = singles.tile([P, 1], f32)
        mbias_t = singles.tile([P, 1], f32)
        nc.vector.memset(mscale_t, -60.0)
        nc.vector.memset(mbias_t, 60.0 * (thr_f + 0.5))
        nc.vector.memset(warm_t, 0.0)

        def issue_sc(sc):
            """Enqueue super-chunk sc's transfers; byte-balance the two HWDGE
            rings while biasing ISSUE work away from the Scalar engine (which
            also runs the mask sigmoids)."""
            if sc == 0:
                # primer: small transfers issued in strict consumption order
                # so the first masks/matmuls never wait behind bytes that are
                # only needed later
                xs0 = slice(0, 2 * P)
                nc.sync.dma_start(out=x_t[:, xs0], in_=xsb_d[:, xs0])
                td_a = tdp.tile([P, 2 * ni], td_dt, tag="td")
                nc.sync.dma_start(out=td_a, in_=tdP_d[0:P, 0 : 2 * ni])
                adj_a = adjp.tile([P, 2 * ni], adj_dt, tag="adj")
                nc.scalar.dma_start(out=adj_a, in_=adjP_d[0:P, 0 : 2 * ni])
                td_b1 = tdp.tile([P, 2 * ni], td_dt, tag="td")
                nc.sync.dma_start(out=td_b1, in_=tdP_d[0:P, 2 * ni : 4 * ni])
                xs1 = slice(2 * P, 4 * P)
                nc.sync.dma_start(out=x_t[:, xs1], in_=xsb_d[:, xs1])
                adj_b = adjp.tile([P, 2 * ni], adj_dt, tag="adj")
                nc.scalar.dma_start(out=adj_b, in_=adjP_d[0:P, 2 * ni : KA * ni])
                td_b2 = tdp.tile([P, 4 * ni], td_dt, tag="td")
                nc.sync.dma_start(out=td_b2, in_=tdP_d[0:P, 4 * ni : KT * ni])
                xs2 = slice(4 * P, KT * P)
                nc.sync.dma_start(out=x_t[:, xs2], in_=xsb_d[:, xs2])
                adj_c = adjp.tile([P, KA * ni], adj_dt, tag="adj")
                nc.scalar.dma_start(out=adj_c, in_=adjP_d[P : 2 * P, :])
                # warm the sigmoid + relu ACT tables while the primer
                # transfers run
                nc.scalar.activation(
                    warm_t, warm_t, mybir.ActivationFunctionType.Sigmoid,
                    bias=mbias_t, scale=mscale_t,
                )
                nc.scalar.activation(
                    warm_t, warm_t, mybir.ActivationFunctionType.Relu,
                    bias=mbias_t, scale=mscale_t,
                )
                return (td_a, td_b1, td_b2, adj_a, adj_b, adj_c)
            a0 = adjp.tile([P, KA * ni], adj_dt, tag="adj")
            q0 = nc.sync if sc % 2 == 0 else nc.scalar
            q0.dma_start(
                out=a0,
                in_=adjP_d[(sc * na_per_sc) * P : (sc * na_per_sc + 1) * P, :],
            )
            td_t = tdp.tile([P, KT * ni], td_dt, tag="td")
            nc.sync.dma_start(out=td_t, in_=tdP_d[sc * P : (sc + 1) * P, :])
            r1 = slice((sc * na_per_sc + 1) * P, (sc * na_per_sc + 2) * P)
            if sc == n_sc - 1:
                # final super-chunk: 2-tile pieces so the drain after the
                # last bytes land is as short as possible
                a1a = adjp.tile([P, 2 * ni], adj_dt, tag="adj")
                nc.scalar.dma_start(out=a1a, in_=adjP_d[r1, 0 : 2 * ni])
                a1b = adjp.tile([P, 2 * ni], adj_dt, tag="adj")
                nc.scalar.dma_start(out=a1b, in_=adjP_d[r1, 2 * ni : KA * ni])
                a1 = None
            else:
                a1 = adjp.tile([P, KA * ni], adj_dt, tag="adj")
                nc.scalar.dma_start(out=a1, in_=adjP_d[r1, :])
                a1a = a1b = None
            xs = slice(sc * KT * P, (sc + 1) * KT * P)
            nc.sync.dma_start(out=x_t[:, xs], in_=xsb_d[:, xs])
            if sc == 1:
                # small constants for the MLP tail, off the critical path
                nc.scalar.dma_start(out=bd1_t, in_=bd1_d)
                nc.scalar.dma_start(out=bd2_t, in_=bd2_d)
                nc.scalar.dma_start(out=bias1_t, in_=bias1_d)
                nc.scalar.dma_start(out=bias2_t, in_=bias2_d)
            if a1 is None:
                return (td_t, [(a0, 0), (a0, 2), (a1a, 0), (a1b, 0)])
            return (td_t, [(a0, 0), (a0, 2), (a1, 0), (a1, 2)])

        # Two mask paths, balanced so neither engine serializes:
        #  - DVE path: fused (td <= thr) * adj scalar_tensor_tensor
        #    (1x mode - the int8 operand blocks 2x);
        #  - ACT path: m = 1[td <= thr] as a saturated sigmoid on the
        #    Scalar engine (min |60*(thr+0.5-td)| = 30 so the fp16 result
        #    is exactly 0/1), then an all-fp16 DVE multiply in 2x mode.
        def emit_mms(a_t, jt0, ntiles):
                for s in range(ntiles):
                    jt = jt0 + s
                    lhsT = x_t[:, jt * P : (jt + 1) * P]
                    for h in range(nh):
                        nc.tensor.matmul(
                            psum_main[:, h * mm_n : (h + 1) * mm_n],
                            lhsT,
                            a_t[:, s * ni + h * mm_n : s * ni + (h + 1) * mm_n],
                            start=(jt == 0),
                            stop=(jt == jt_n - 1),
                        )

        def stt_pair(td_sl, adj_sl, jt0, ntiles):
            a_t = wrk.tile([P, ntiles * ni], mm_dt, tag="a")
            nc.vector.scalar_tensor_tensor(
                a_t, td_sl, thr_f, adj_sl,
                op0=mybir.AluOpType.is_le,
                op1=mybir.AluOpType.mult,
            )
            emit_mms(a_t, jt0, ntiles)

        def act_pair(td_sl, adj_sl, jt0):
            m_t = mp.tile([P, 2 * ni], mm_dt, tag="m")
            nc.scalar.activation(
                m_t, td_sl,
                mybir.ActivationFunctionType.Sigmoid,
                bias=mbias_t, scale=mscale_t,
            )
            a_t = wrk.tile([P, 2 * ni], mm_dt, tag="a")
            nc.vector.tensor_tensor(
                a_t, m_t, adj_sl, op=mybir.AluOpType.mult
            )
            emit_mms(a_t, jt0, 2)

        def do_pair(td_sl, adj_sl, jt0):
            # [STT, ACT, ACT] rotation: the 2x-mode ACT-path multiply is
            # ~2x cheaper on the DVE than the fused STT, so weighting
            # toward it lowers the DVE's steady-state cadence; deep mp/wrk
            # pools let the sigmoids run ahead so consecutive ACT pairs
            # don't stall the DVE.  The sigmoid compare is only exact for
            # 1-byte td (|scale*td| stays small), so wide td falls back
            # to all-STT.
            if pair_ctr[0] % 3 == 0 or np.dtype(td_dtype).itemsize > 1:
                stt_pair(td_sl, adj_sl, jt0, 2)
            else:
                act_pair(td_sl, adj_sl, jt0)
            pair_ctr[0] += 1

        def pe_keepwarm(n):
            # dummy matmuls into a scratch PSUM bank during known DMA-wait
            # windows: the PE only reaches its 2.4 GHz p-state after ~3us of
            # CONTINUOUS execution, and the early-stream stalls otherwise
            # reset it to 1.2 GHz for the whole run.  Sized below the
            # measured stall lengths so they never delay real work.
            for _ in range(n):
                nc.tensor.matmul(
                    scr_ps, x_t[:, 0:P], x_t[:, 0:mm_n], start=True, stop=True
                )

        # software-pipeline the DMA issue one super-chunk ahead of the mask
        # emission: the Scalar ring otherwise drains while its next
        # dma_start sits behind sem-blocked sigmoids in the engine queue
        keepwarm = {0: 16, 1: 24, 2: 8}
        chunks = {0: issue_sc(0)}
        for sc in range(n_sc):
            if sc + 1 < n_sc:
                chunks[sc + 1] = issue_sc(sc + 1)
            ch = chunks.pop(sc)
            if sc == 0:
                td_a, td_b1, td_b2, adj_a, adj_b, adj_c = ch
                # primer: single-tile fused masks so the PE starts as early
                # as possible
                stt_pair(td_a[:, 0:ni], adj_a[:, 0:ni], 0, 1)
                stt_pair(td_a[:, ni : 2 * ni], adj_a[:, ni : 2 * ni], 1, 1)
                pair_ctr[0] += 1
                do_pair(td_b1[:, :], adj_b[:, :], 2)
                do_pair(td_b2[:, 0 : 2 * ni], adj_c[:, 0 : 2 * ni], 4)
                do_pair(td_b2[:, 2 * ni : 4 * ni], adj_c[:, 2 * ni : 4 * ni], 6)
            else:
                td_t, adj_list = ch
                act_ok = np.dtype(td_dtype).itemsize == 1
                for q in range(KT // 2):
                    at, off = adj_list[q]
                    td_sl = td_t[:, 2 * q * ni : (2 * q + 2) * ni]
                    adj_sl = at[:, off * ni : (off + 2) * ni]
                    jt0 = sc * KT + 2 * q
                    force_act = act_ok and (
                        sc == n_sc - 1 or (sc == n_sc - 2 and q >= 2)
                    )
                    if force_act:
                        # final super-chunk: all-ACT masks.  The sigmoids
                        # precompute on the Scalar engine while the DVE is
                        # still on the previous chunk, so the post-last-byte
                        # drain runs at the cheap 2x-multiply cadence.
                        act_pair(td_sl, adj_sl, jt0)
                    else:
                        do_pair(td_sl, adj_sl, jt0)
            pe_keepwarm(keepwarm.get(sc, 0))

        # Per-node MLP, pipelined in independent column quarters.
        h_ps = pp.tile([P, ni], f32, tag="hps")
        o_ps = pp.tile([P, ni], f32, tag="ops")
        nq = 4
        qn = ni // nq
        for h in range(nq):
            hs = slice(h * qn, (h + 1) * qn)
            res_t = singles.tile([P, qn], mm_dt, tag=f"res{h}", name=f"res{h}")
            nc.vector.tensor_copy(res_t, psum_main[:, hs])
            nc.tensor.matmul(h_ps[:, hs], bd1_t, res_t, start=True, stop=True)
            # h = relu(. + b1) on the ACT engine - takes a DVE stage off the
            # tail chain (the DVE is still finishing the last mask multiplies)
            h_t = singles.tile([P, qn], mm_dt, tag=f"h{h}", name=f"h{h}")
            nc.scalar.activation(
                h_t, h_ps[:, hs], mybir.ActivationFunctionType.Relu,
                bias=bias1_t,
            )
            nc.tensor.matmul(o_ps[:, hs], bd2_t, h_t, start=True, stop=True)
            out_t = singles.tile([P, qn], out_dt, tag=f"out{h}", name=f"out{h}")
            nc.scalar.activation(
                out_t, o_ps[:, hs], mybir.ActivationFunctionType.Sigmoid, bias=bias2_t
            )
            nc.sync.dma_start(out=outT_d[:, hs], in_=out_t)

    nc.compile()
    return nc


def _chunk_pack(arr2d, k, p, ni):
    """[jt_n*P, ni] -> [(jt_n/k)*P, k*ni] with each P-row block one
    contiguous DMA chunk of k contraction tiles (layout-only)."""
    jt_n = arr2d.shape[0] // p
    return np.ascontiguousarray(
        arr2d.reshape(jt_n // k, k, p, ni).transpose(0, 2, 1, 3)
        .reshape((jt_n // k) * p, k * ni)
    )


def _host_prep(x, adj, time_delay, t, W1, b1, W2, b2, ncores, mm_np, td_dtype):
    """Layout-only transforms (transpose / repack / dtype container changes)."""
    x = np.ascontiguousarray(np.asarray(x, dtype=np.float32))
    adj = np.asarray(adj, dtype=np.float32)
    td = np.asarray(time_delay)
    b, n, f = x.shape
    ni = n // ncores
    jt_n = n // P

    thr = int(t) * 5 + 4  # time_delay // 5 <= t  <=>  time_delay <= 5t+4

    adj_np = mm_np if mm_np == np.float16 else np.float32
    adjT = np.ascontiguousarray(adj.T.astype(adj_np))
    tdT = np.ascontiguousarray(td.T.astype(td_dtype))
    # stationary x: x_sb[p, jt*P + 32*b + f] = x[b, jt*P + p, f]
    xsb = np.ascontiguousarray(
        x.reshape(b, jt_n, P, f).transpose(2, 1, 0, 3).reshape(P, jt_n * b * f)
        .astype(mm_np)
    )
    bd1 = np.zeros((P, P), np.float32)
    bd2 = np.zeros((P, P), np.float32)
    for bb in range(b):
        bd1[bb * f : (bb + 1) * f, bb * f : (bb + 1) * f] = W1
        bd2[bb * f : (bb + 1) * f, bb * f : (bb + 1) * f] = W2
    bd1 = np.ascontiguousarray(bd1.astype(mm_np))
    bd2 = np.ascontiguousarray(bd2.astype(mm_np))
    bias1 = np.ascontiguousarray(np.tile(np.asarray(b1, np.float32), b).reshape(P, 1))
    bias2 = np.ascontiguousarray(np.tile(np.asarray(b2, np.float32), b).reshape(P, 1))

    in_maps = []
    for c in range(ncores):
        sl = slice(c * ni, (c + 1) * ni)
        in_maps.append(
            {
                "adjP": _chunk_pack(adjT[:, sl], KA, P, ni),
                "tdP": _chunk_pack(tdT[:, sl], KT, P, ni),
                "xsb": xsb,
                "bd1": bd1,
                "bd2": bd2,
                "bias1": bias1,
                "bias2": bias2,
            }
        )
    return thr, in_maps


def _run(x, adj, time_delay, t, W1, b1, W2, b2, ncores=NCORES,
         mm_dtype_name="float16", trace=False):
    from concourse.bass_utils import run_bass_kernel_spmd

    b, n, f = np.asarray(x).shape
    ni = n // ncores
    td = np.asarray(time_delay)
    # int8 shipping is only a container change; keep int32 when values
    # (or the threshold compare range) would not fit exactly.
    thr_chk = int(t) * 5 + 4
    if td.min() >= -127 and td.max() <= 127 and -127 <= thr_chk <= 127:
        td_dtype = np.int8
    else:
        td_dtype = np.int32
    mm_np = np.float16 if mm_dtype_name == "float16" else np.float32
    thr, in_maps = _host_prep(
        x, adj, time_delay, t, W1, b1, W2, b2, ncores, mm_np, td_dtype
    )
    nc = _build(n, ni, thr, mm_dtype_name, td_dtype)
    res = run_bass_kernel_spmd(
        nc, in_maps, core_ids=list(range(ncores)), trace=trace
    )
    full = np.concatenate(
        [r["outT"].astype(np.float32) for r in res.results], axis=1
    )  # [P, n]
    out = np.ascontiguousarray(full.reshape(b, f, n).transpose(0, 2, 1))
    return out, res


def kernel(x, adj, time_delay, t, W1, b1, W2, b2):
    out, _ = _run(x, adj, time_delay, t, W1, b1, W2, b2)
    return out
